# revision 32
# baseline (speedup 1.0000x reference)
"""AnoNAViLa forward kernel for 8 TRN2 NeuronCores (data-parallel over batch).

Math (per branch):
  sims = (img @ text.T) * scale;  w = softmax(sims);  e = exp(w)
  x = concat([img_rep, text * e[..., None]], -1)
  h = relu(x @ W1 + b1); h = relu(h @ W2 + b2); h = h @ W3 + b3
  out = h.mean(axis=1)

Key algebraic restructuring (exact, up to fp assoc):
  x @ W1 = img @ W1[:D] + e[b,n] * (text @ W1[D:])      (rank-1 per (b,n))
  mean_n (h2 @ W3 + b3) = (mean_n h2) @ W3 + b3          (mean before layer 3)

Layer-1 is emitted as ONE fp8 DoubleRow matmul per 128-chunk of d, packing
three things into the 256 contraction rows (a_bd = img @ W1_top, untransposed):
  tile0 rows 0..3   : lhsT = tW quad rows (rank-4 selector), rhs = block-diag e
  tile0 rows 4..127 : lhsT = a_res columns (group-rotated),  rhs = delta pattern
  tile1 rows 0..127 : lhsT = a_hi columns,                   rhs = lane-replicated I
  => psum = e (x) tW + (a_hi + a_res)^T    (no identity-broadcast matmul)
a is fp8 hi+residual compensated (single-fp8 a alone costs 1.7e-2 rel err —
correlated across n).  The residual covers 124 of 128 batch columns; the
uncovered 4 rotate per quad-group so each batch row misses its residual in at
most 1 of 24 groups.  Net error ~5e-3 against the 2e-2 gate; layer 2 stays
bf16.
"""
import sys

sys.path.insert(0, "/opt/trn_rl_repo")

from contextlib import ExitStack

import numpy as np
import orjson

import concourse.bass as bass
import concourse.mybir as mybir
import concourse.tile as tile
from concourse.bass import ds, ts
from concourse.bass_utils import run_bass_kernel_spmd

F32 = mybir.dt.float32
BF16 = mybir.dt.bfloat16
F8 = mybir.dt.float8e4
AF = mybir.ActivationFunctionType
ALU = mybir.AluOpType
PM = mybir.MatmulPerfMode

NC = 8
B, N, D = 1024, 96, 512
BL = B // NC  # 128 rows per core
NG = N // 4  # 24 quad groups

SEL_EXT = NG * 4 * 2 * 128  # selreg per-partition extent (fp8 elems)
ERG_EXT = NG * 2 * 512      # ereg per-partition extent


# ---------------------------------------------------------------------------
# This walrus build rejects instructions with >1 semaphore wait/update
# ("Too many sync wait commands").  Split extras onto chained NoOps on the
# same engine (streams are in-order, so sequential waits == combined wait).
_bir_patch_installed = False


def _split_multi_sync(bir_json: bytes) -> bytes:
    d = orjson.loads(bir_json)
    ctr = [0]

    def mk_nop(inst, wait=None, update=None):
        ctr[0] += 1
        return {
            "debug": inst.get("debug", 0),
            "engine": inst["engine"],
            "ins": [],
            "outs": [],
            "name": f"{inst['name']}__ssplit{ctr[0]}",
            "opcode": "NoOp",
            "sync_info": {
                "on_update": [update] if update else [],
                "on_wait": [wait] if wait else [],
            },
        }

    changed = False
    for fn in d["functions"]:
        for bb in fn["blocks"]:
            new_insts = []
            for inst in bb["instructions"]:
                si = inst.get("sync_info")
                pre, post = [], []
                if si:
                    waits = si.get("on_wait") or []
                    if len(waits) > 1:
                        pre = [mk_nop(inst, wait=w) for w in waits[:-1]]
                        si["on_wait"] = [waits[-1]]
                        changed = True
                    upds = si.get("on_update") or []
                    if len(upds) > 1:
                        post = [mk_nop(inst, update=u) for u in upds[1:]]
                        si["on_update"] = [upds[0]]
                        changed = True
                new_insts.extend(pre)
                new_insts.append(inst)
                new_insts.extend(post)
            bb["instructions"] = new_insts
    return orjson.dumps(d) if changed else bir_json


def _install_bir_patch():
    global _bir_patch_installed
    if _bir_patch_installed:
        return
    _bir_patch_installed = True
    import concourse.bass_utils as bu
    import concourse.bass2jax as b2j

    orig = bu.compile_bir_kernel

    def patched(bir_json, tmpdir, neff_name="file.neff"):
        return orig(_split_multi_sync(bir_json), tmpdir, neff_name)

    bu.compile_bir_kernel = patched
    b2j.compile_bir_kernel = patched


def _flat(ap, dims, extra_off=0):
    """AP over the same tensor with explicit flat-element dims."""
    return bass.AP(tensor=ap.tensor, offset=ap.offset + extra_off, ap=dims)


# ---------------------------------------------------------------------------
def build_graph(scale: float, has_b1: bool, has_b2: bool, has_b3: bool) -> bass.Bass:
    nc = bass.Bass()

    # host-prepared layouts: [p, c, ...] with p the SBUF partition
    imgT_ext = nc.declare_dram_parameter("imgt", [128, 4, BL], F32, isOutput=False)
    tnT_ext = nc.declare_dram_parameter("tnt", [128, 4, N], F32, isOutput=False)
    taT_ext = nc.declare_dram_parameter("tat", [128, 4, N], F32, isOutput=False)
    W1_ext = nc.declare_dram_parameter("w1bf", [128, 8, D], BF16, isOutput=False)
    W2_ext = nc.declare_dram_parameter("w2bf", [128, 4, D // 2], BF16, isOutput=False)
    W3_ext = nc.declare_dram_parameter("w3bf", [128, 2, D // 4], BF16, isOutput=False)
    I4_ext = nc.declare_dram_parameter("i4", [128, 512], F8, isOutput=False)
    patt_ext = nc.declare_dram_parameter("patt", [128, NG, 512], F8, isOutput=False)
    if has_b1:
        b1_ext = nc.declare_dram_parameter("b1row", [1, D], BF16, isOutput=False)
    if has_b2:
        b2_ext = nc.declare_dram_parameter("b2row", [1, D // 2], F32, isOutput=False)
    if has_b3:
        b3_ext = nc.declare_dram_parameter("b3t", [128, 1], F32, isOutput=False)
    out_ext = nc.declare_dram_parameter("out", [2, D // 4, BL], F32, isOutput=True)

    with tile.TileContext(nc) as tc, ExitStack() as ctx:
        const = ctx.enter_context(tc.tile_pool(name="const", bufs=1))
        work = ctx.enter_context(tc.tile_pool(name="work", bufs=2))
        hpool = ctx.enter_context(tc.tile_pool(name="hpool", bufs=4))
        psH1 = ctx.enter_context(tc.tile_pool(name="psH1", bufs=2, space="PSUM"))
        psH2 = ctx.enter_context(tc.tile_pool(name="psH2", bufs=2, space="PSUM"))

        # ---- fused layer-1 operand regions (per branch) --------------------
        # selreg[p, g, c, i, m]: i=0 -> rows 0..3 tW quad, rows 4..127 a_res
        # columns rotated by 4g; i=1 -> a_hi columns, replicated per group.
        # ereg[p, g, i, f]:      i=0 -> rows 0..3 block-diag e, rows 4..127
        # host delta pattern matching the rotation; i=1 -> I4.
        # Every byte is written by a DMA below, so no memsets are needed.
        selregs = [const.tile([128, NG, 4, 2, 128], F8, name=f"selreg{br}")
                   for br in range(2)]
        eregs = [const.tile([128, NG, 2, 512], F8, name=f"ereg{br}")
                 for br in range(2)]

        # ---- DMA loads (host already did all transposes/casts) -------------
        imgT_f = const.tile([128, 4, BL], F32)
        nc.sync.dma_start(out=imgT_f[:], in_=imgT_ext[:, :, :])
        tnT_s = const.tile([128, 4, N], F32)
        nc.scalar.dma_start(out=tnT_s[:], in_=tnT_ext[:, :, :])
        taT_s = const.tile([128, 4, N], F32)
        nc.scalar.dma_start(out=taT_s[:], in_=taT_ext[:, :, :])
        I4_s = const.tile([128, 512], F8)
        nc.gpsimd.dma_start(out=I4_s[:], in_=I4_ext[:, :])

        W1bf = const.tile([128, 8, D], BF16)
        nc.scalar.dma_start(out=W1bf[:, 0:4, :], in_=W1_ext[:, 0:4, :])
        nc.scalar.dma_start(out=W1bf[:, 4:6, :], in_=W1_ext[:, 4:6, :])
        nc.sync.dma_start(out=W1bf[:, 6:8, :], in_=W1_ext[:, 6:8, :])
        W2_bf = const.tile([128, 4, D // 2], BF16)
        nc.gpsimd.dma_start(out=W2_bf[:], in_=W2_ext[:, :, :])
        W3bf = const.tile([128, 2, D // 4], BF16)
        nc.gpsimd.dma_start(out=W3bf[:], in_=W3_ext[:, :, :])
        if has_b1:
            b1row = const.tile([1, D], BF16)
            nc.gpsimd.dma_start(out=b1row[:], in_=b1_ext[:, :])
            ones_bf = const.tile([1, 128], BF16)
            nc.gpsimd.memset(ones_bf[:], 1.0)
        if has_b2:
            b2row = const.tile([1, D // 2], F32)
            nc.gpsimd.dma_start(out=b2row[:], in_=b2_ext[:, :])
            b2row_bf = const.tile([1, D // 2], BF16)
            nc.gpsimd.tensor_copy(b2row_bf[:], b2row[:])
            ones2_bf = const.tile([1, D], BF16)
            nc.gpsimd.memset(ones2_bf[:], 1.0)
        if has_b3:
            b3t = const.tile([128, 1], F32)
            nc.gpsimd.dma_start(out=b3t[:], in_=b3_ext[:, :])

        imgT_bf = const.tile([128, 4, BL], BF16)
        nc.scalar.copy(imgT_bf[:], imgT_f[:])

        # delta-pattern rows (incl. zero rows 0..3) + I4 into each ereg;
        # branch-0's patt on sync, branch-1's on scalar, I4 on gpsimd
        for br in range(2):
            peng = nc.sync if br == 0 else nc.scalar
            peng.dma_start(out=eregs[br][:, :, 0, :], in_=patt_ext[:, :, :])
            src = I4_s[:]
            nc.gpsimd.dma_start(
                out=eregs[br][:, :, 1, :],
                in_=bass.AP(tensor=src.tensor, offset=src.offset,
                            ap=[src.ap[0], [0, NG], [1, 512]]),
            )

        # ---- per-branch prologue, phase A: softmax chain -> e scatter ------
        def prologue_eA(br, textT_s):
            textT_bf = work.tile([128, 4, N], BF16, tag="textT_bf")
            nc.scalar.copy(textT_bf[:], textT_s[:])

            # sims = img @ text.T (scale folded into the exp activation)
            ps_sims = psH2.tile([BL, N], F32, tag="h2p")
            for c in range(4):
                nc.tensor.matmul(
                    ps_sims[:], imgT_f[:, c, :], textT_s[:, c, :],
                    start=(c == 0), stop=(c == 3),
                )
            # softmax over n then e = exp(w), all rowwise
            negmax = work.tile([BL, 1], F32, tag="negmax")
            nc.vector.tensor_reduce(
                negmax[:], ps_sims[:], axis=mybir.AxisListType.X, op=ALU.max,
                negate=True,
            )
            nb = work.tile([BL, 1], F32, tag="nb")
            nc.vector.tensor_scalar_mul(nb[:], negmax[:], float(scale))
            E_s = work.tile([BL, N], F32, tag="E_s")
            ssum = work.tile([BL, 1], F32, tag="ssum")
            nc.scalar.activation(
                E_s[:], ps_sims[:], AF.Exp, bias=nb[:, 0:1], scale=float(scale),
                accum_out=ssum[:, 0:1],
            )
            rr = work.tile([BL, 1], F32, tag="rr")
            nc.vector.reciprocal(rr[:], ssum[:])
            e_x = work.tile([BL, 128], BF16, tag="e_x")
            nc.scalar.activation(e_x[:, 0:N], E_s[:], AF.Exp, scale=rr[:, 0:1])

            # eT via DMA transpose (2-byte dtype, padded to 128 cols), then
            # fp8-convert and scatter block-diagonals into ereg rows 0..3.
            # Branch-1's chain stays on the scalar queue so it can't block
            # branch-0-critical work queued on sync (deferred tW scatters).
            dma_eng = nc.sync if br == 0 else nc.scalar
            eT_bf = work.tile([128, BL], BF16, tag="eT_bf")
            dma_eng.dma_start(out=eT_bf[:], in_=e_x[:], transpose=True)
            eT8 = work.tile([128, BL], F8, tag="eT8")
            nc.vector.tensor_copy(eT8[:], eT_bf[:])
            dstE = eregs[br][:]
            srcE = eT8[:]
            for k in range(4):
                dma_eng.dma_start(
                    out=_flat(dstE, [[ERG_EXT, 1], [2 * 512, NG], [1, 128]],
                              extra_off=k * ERG_EXT + k * 128),
                    in_=_flat(srcE, [[4 * 128, NG], [1, 128]],
                              extra_off=k * 128),
                )
            return textT_bf

        tW8s = [None, None]

        def prologue_tw(br, textT_bf):
            # tW = text @ W1_bot -> fp8 (scatters into selreg are deferred)
            ptw = psH2.tile([N, D], F32, tag="h2p")
            for c in range(4):
                nc.tensor.matmul(
                    ptw[:], textT_bf[:, c, :], W1bf[:, 4 + c, :],
                    start=(c == 0), stop=(c == 3),
                )
            tW8 = work.tile([N, D], F8, tag="tW8")
            nc.scalar.activation(tW8[:], ptw[:], AF.Identity)
            tW8s[br] = tW8

        # branch 0's e-chain goes first so its scatters land early
        textT_bf0 = prologue_eA(0, tnT_s)

        # ---- a_bd = img @ W1_top (+ b1): [b, d] then fp8 hi + residual
        pa = psH2.tile([128, D], F32, tag="h2p")
        for ci in range(4):
            nc.tensor.matmul(
                pa[:], imgT_bf[:, ci, :], W1bf[:, ci, :],
                start=(ci == 0), stop=(ci == 3 and not has_b1),
            )
        if has_b1:
            nc.tensor.matmul(
                pa[:], ones_bf[0:1, :], b1row[0:1, :],
                start=False, stop=True, skip_group_check=True,
            )
        a8 = const.tile([128, D], F8)
        nc.scalar.activation(a8[:], pa[:], AF.Identity)
        # fp8 residual underflows e4m3 denormals (res ~1e-3 < 2^-9), so store
        # res*2^6 and put 2^-6 in the delta pattern instead of 1.0
        res_bf = work.tile([128, D], BF16, tag="res_bf")
        nc.vector.tensor_sub(res_bf[:], pa[:], a8[:])
        ar8 = const.tile([128, D], F8)
        nc.scalar.activation(ar8[:], res_bf[:], AF.Identity, scale=64.0)

        # The a-halves of selreg are replicated per group in 128-byte pieces;
        # DMA issue cost for that fragmentation is huge on the issuing
        # engine, so replicate with compute-engine broadcast copies
        # (stride-0 source reads) instead.  Engine ops need aligned partition
        # windows, so the class shifts are pre-applied with cheap
        # partition-shift DMAs into ars[k] (row 4+j = ar8[(j+4k)%128]; rows
        # 0..3 hold wrapped junk that the later tW row-scatter overwrites).
        ars = []
        for k in range(4):
            t = const.tile([128, D], F8, tag=f"ars{k}", name=f"ars{k}")
            p = 0
            while p < 128:
                sa = (p - 4 + 4 * k) % 128
                na = min(128 - p, 128 - sa)
                eng = nc.sync if k == 0 else nc.gpsimd
                eng.dma_start(out=t[p:p + na, :], in_=ar8[sa:sa + na, :])
                p += na
            ars.append(t)

        def _bcopy(eng, out, in_):
            if eng is nc.scalar:
                eng.copy(out, in_)
            else:
                eng.tensor_copy(out, in_)

        def emit_ahi(br, k, c, eng):
            # a_hi columns of chunk c (k=None: all groups, else 6k..6k+5)
            gsl = slice(0, NG) if k is None else slice(6 * k, 6 * k + 6)
            ng = NG if k is None else 6
            src = a8[:, ts(c, 128)]
            _bcopy(
                eng,
                selregs[br][:, gsl, c, 1, :],
                bass.AP(tensor=src.tensor, offset=src.offset,
                        ap=[src.ap[0], [0, ng], [1, 128]]),
            )

        def emit_ares(br, k, c, eng):
            # shift class k covers groups 6k..6k+5; writes junk into rows
            # 0..3 which emit_tw overwrites (emission order = dependency)
            src = ars[k][:, ts(c, 128)]
            _bcopy(
                eng,
                selregs[br][:, 6 * k:6 * k + 6, c, 0, :],
                bass.AP(tensor=src.tensor, offset=src.offset,
                        ap=[src.ap[0], [0, 6], [1, 128]]),
            )

        def emit_tw(br, k):
            # tW quad rows into selreg rows 0..3 for groups 6k..6k+5
            dstW = selregs[br][:]
            srcW = tW8s[br][:]
            for r in range(4):
                nc.sync.dma_start(
                    out=_flat(dstW, [[SEL_EXT, 1], [4 * 2 * 128, 6],
                                     [2 * 128, 4], [1, 128]],
                              extra_off=r * SEL_EXT + 6 * k * 1024),
                    in_=_flat(srcW, [[4 * D, 6], [128, 4], [1, 128]],
                              extra_off=r * D + 6 * k * 4 * D),
                )

        # branch-0 operands upfront: full-range a_hi plus the class-0 a_res,
        # spread across scalar/vector (gpsimd's tensor_copy is ~2.5x slower,
        # keep it off the ramp)
        for c, eng in zip(range(4), (nc.scalar, nc.vector, nc.scalar, nc.vector)):
            emit_ahi(0, None, c, eng)
        for c, eng in zip(range(4), (nc.vector, nc.scalar, nc.vector, nc.scalar)):
            emit_ares(0, 0, c, eng)
        # branch-0 tW for groups 0..5
        prologue_tw(0, textT_bf0)
        emit_tw(0, 0)

        # deferred fills, popped between main-loop units; each class's a_res
        # copies precede its tW scatter.  Copies on gpsimd (idle in the main
        # loop; scalar/vector run the evictions), scatters on sync.
        pending = []
        for k in range(1, 4):
            for c in range(4):
                pending.append(lambda k=k, c=c: emit_ares(0, k, c, nc.gpsimd))
            pending.append(lambda k=k: emit_tw(0, k))
        for k in range(4):
            for c in range(4):
                pending.append(lambda k=k, c=c: emit_ahi(1, k, c, nc.gpsimd))
                pending.append(lambda k=k, c=c: emit_ares(1, k, c, nc.gpsimd))
            pending.append(lambda k=k: emit_tw(1, k))

        # branch-1 prologue (e-chain on scalar queue, tW via deferred pops)
        textT_bf1 = prologue_eA(1, taT_s)
        prologue_tw(1, textT_bf1)

        maccs = [
            const.tile([128, 2 * D], F32, tag=f"macc4_{i}", name=f"macc4_{i}")
            for i in range(2)
        ]
        macc_bfs = [
            const.tile([128, 2 * D], BF16, tag=f"maccbf_{i}", name=f"maccbf_{i}")
            for i in range(2)
        ]

        def emit_layer3(br):
            # fold the 4 n-lanes inside the layer-3 matmul (K-accumulation),
            # so no vector fold sits on the critical path
            macc_bf = macc_bfs[br]
            po = psH1.tile([128, 128], F32, tag="h1p")
            for m in range(2):
                for q in range(4):
                    nc.tensor.matmul(
                        po[:], W3bf[:, m, :], macc_bf[:, ds(m * D + q * 128, 128)],
                        start=(m == 0 and q == 0), stop=(m == 1 and q == 3),
                        skip_group_check=True,
                    )
            outT = work.tile([128, 128], F32, tag="outT")
            if has_b3:
                nc.vector.tensor_scalar(
                    out=outT[:], in0=po[:], scalar1=1.0 / N, scalar2=b3t[:, 0:1],
                    op0=ALU.mult, op1=ALU.add,
                )
            else:
                nc.scalar.activation(outT[:], po[:], AF.Identity, scale=1.0 / N)
            nc.sync.dma_start(out=out_ext[br, :, :], in_=outT[:])

        # ---- main loop: software-pipelined one unit deep --------------------
        # Unit (br,g)'s four DoubleRows are emitted, then the PREVIOUS unit's
        # layer-2 + macc.  By the time the PE reaches that layer-2, its h1
        # operands were evicted during this unit's DoubleRows, so the PE
        # never waits on an eviction.
        def emit_l2(br, g, h1a, h1b):
            macc4 = maccs[br]
            ph2 = psH2.tile([128, 2 * D], F32, tag="h2p")
            if has_b2:
                for m in range(2):
                    nc.tensor.matmul(
                        ph2[:, ds(m * D, D)],
                        b2row_bf[0:1, ts(m, 128)], ones2_bf[0:1, :],
                        start=True, stop=True,
                    )
            for c in range(4):
                h1x = h1a if c < 2 else h1b
                for m in range(2):
                    nc.tensor.matmul(
                        ph2[:, ds(m * D, D)],
                        W2_bf[:, c, ts(m, 128)],
                        h1x[:, ds((c % 2) * D, D)],
                        start=(c == 0 and not has_b2), stop=(c == 3),
                        skip_group_check=True,
                    )
            # fused: macc4 += relu(ph2); first group initializes, last
            # group emits bf16 directly for the bf16 layer-3 matmul
            if g == 0:
                nc.vector.tensor_scalar_max(macc4[:], ph2[:], 0.0)
            elif g == NG - 1:
                nc.vector.scalar_tensor_tensor(
                    out=macc_bfs[br][:], in0=ph2[:], scalar=0.0,
                    in1=macc4[:], op0=ALU.max, op1=ALU.add,
                )
            else:
                nc.vector.scalar_tensor_tensor(
                    out=macc4[:], in0=ph2[:], scalar=0.0,
                    in1=macc4[:], op0=ALU.max, op1=ALU.add,
                )

        prev = None
        for br in range(2):
            for g in range(NG):
                for _ in range(1 if br == 0 else 3):
                    if pending:
                        pending.pop(0)()  # trickle deferred selreg fills
                if br == 1 and g == 3:
                    emit_layer3(0)  # branch-0 output, tucked into the loop
                h1a = hpool.tile([128, 2 * D], BF16, tag="h1a")
                h1b = hpool.tile([128, 2 * D], BF16, tag="h1b")
                for pair, h1x in ((0, h1a), (1, h1b)):
                    ph1 = psH1.tile([128, 2 * D], F32, tag="h1p")
                    for ci in range(2):
                        c = 2 * pair + ci
                        # one DoubleRow: e (x) tW  +  a_hi + a_res
                        nc.tensor.matmul(
                            ph1[:, ds(ci * D, D)],
                            selregs[br][:, g, c, :, :], eregs[br][:, g, :, :],
                            start=True, stop=True, perf_mode=PM.DoubleRow,
                        )
                    # relu evictions split across Scalar (pair 0) / DVE (pair 1)
                    if pair == 0:
                        nc.scalar.activation(h1x[:], ph1[:], AF.Relu)
                    else:
                        nc.vector.tensor_scalar_max(h1x[:], ph1[:], 0.0)
                if prev is not None:
                    emit_l2(*prev)
                prev = (br, g, h1a, h1b)

        emit_l2(*prev)
        emit_layer3(1)

    return nc


def make_in_maps(inputs):
    import ml_dtypes

    BF = ml_dtypes.bfloat16
    F8NP = ml_dtypes.float8_e4m3
    f32 = np.float32

    def rearr_w(w, p=128):
        # [C*p, d] -> [p, C, d]
        cpd = np.asarray(w, f32)
        c = cpd.shape[0] // p
        return np.ascontiguousarray(cpd.reshape(c, p, -1).transpose(1, 0, 2))

    def rearr_t(x):
        # [n, 4*128] -> [128, 4, n]  (transposed, chunked)
        xt = np.asarray(x, f32).T  # [512, n]
        return np.ascontiguousarray(xt.reshape(4, 128, -1).transpose(1, 0, 2))

    # delta pattern: row 4+j of group g routes 64*a_res[(j+s)%128] (s=4*(g//6))
    # to batch column (j+s)%128 in every lane block, with weight 2^-6 to undo
    # the residual scaling; rows 0..3 stay zero (e rows)
    patt = np.zeros((128, NG, 512), f32)
    j = np.arange(124)
    for g in range(NG):
        b = (j + 4 * (g // 6)) % 128
        for k in range(4):
            patt[4 + j, g, k * 128 + b] = 1.0 / 64.0

    img = np.asarray(inputs["img_embs"], f32)
    b1 = np.asarray(inputs["b1"], f32)
    b2 = np.asarray(inputs["b2"], f32)
    b3 = np.asarray(inputs["b3"], f32)
    shared = {
        "patt": patt.astype(F8NP),
        "tnt": rearr_t(inputs["normal_text_embs"]),
        "tat": rearr_t(inputs["abnormal_text_embs"]),
        "w1bf": rearr_w(inputs["W1"]).astype(BF),
        "w2bf": rearr_w(inputs["W2"]).astype(BF),
        "w3bf": rearr_w(inputs["W3"]).astype(BF),
        "i4": np.ascontiguousarray(np.tile(np.eye(128, dtype=f32), (1, 4))).astype(F8NP),
    }
    if np.any(b1):
        shared["b1row"] = np.ascontiguousarray(b1.reshape(1, -1)).astype(BF)
    if np.any(b2):
        shared["b2row"] = np.ascontiguousarray(b2.reshape(1, -1))
    if np.any(b3):
        shared["b3t"] = np.ascontiguousarray(b3.reshape(-1, 1))
    return [dict(shared, imgt=rearr_t(img[i * BL : (i + 1) * BL])) for i in range(NC)]


def kernel(**inputs) -> tuple:
    _install_bir_patch()

    scale = float(np.exp(np.asarray(inputs["logit_scale"], np.float32).reshape(-1)[0]))
    has_b1 = bool(np.any(np.asarray(inputs["b1"], np.float32)))
    has_b2 = bool(np.any(np.asarray(inputs["b2"], np.float32)))
    has_b3 = bool(np.any(np.asarray(inputs["b3"], np.float32)))

    nc = build_graph(scale, has_b1, has_b2, has_b3)
    in_maps = make_in_maps(inputs)
    res = run_bass_kernel_spmd(nc, in_maps, core_ids=list(range(NC)))
    h_n = np.concatenate([res.results[i]["out"][0].T for i in range(NC)], axis=0)
    h_a = np.concatenate([res.results[i]["out"][1].T for i in range(NC)], axis=0)
    return (h_n, h_a)
